# revision 20
# baseline (speedup 1.0000x reference)
"""Causal self-attention kernel for 8 Trainium2 NeuronCores.

Problem: B=4, T=2048, C=1024, H=16 heads (D=64).
Sharding: data-parallel over batch (4) x tensor-parallel over heads (2 groups
of 8 heads). Core c handles batch c//2, head-group c%2. Each core computes
qkv for its 8 heads, full causal attention on TxT scores, and its partial
projection output; the host sums the two head-group partials per batch.

v2 design notes (vs the v1 baseline at 382us):
  - chunk-outer pipeline: for each 512-wide i-chunk, all 4 head pairs run
    attention back to back while "filler" matmuls (next chunk's v/q/k, and
    deferred proj tiles) are pumped into the PE stream between attention
    steps. This keeps the tensor engine's HAM clock-gate warm (2.4 GHz needs
    >3.4us of continuous PE activity) and hides QKV/proj almost entirely
    under the exp() stream on the scalar engine, which is the hard floor
    (~150us of exp work per core).
  - all input DMAs are issued up front across three queues (gpsimd for
    weights, sync+scalar for the 8 x-halves) so compute never starves.
  - softmax denominator: v carries a ones column (PSUM row 64 = l partial);
    1/l via reciprocal_approx_fast straight off PSUM (no DMA lane-spread
    round trips), PE ones-column broadcast, in-place DVE multiply.
  - scores for a pair are computed in one [128,2,512] PSUM tile via the
    64-row tile_position trick (two matmuls co-execute).
  - proj outputs staged bf16 and DMA'd on gpsimd/sync rings; host sums the
    two head-group partials in fp32.
"""

import sys

if "/opt/trn_rl_repo" not in sys.path:
    sys.path.insert(0, "/opt/trn_rl_repo")

from collections import deque
from contextlib import ExitStack

import ml_dtypes
import numpy as np

import concourse.bass as bass
import concourse.mybir as mybir
import concourse.tile as tile
from concourse.bass_utils import run_bass_kernel_spmd

BF16 = mybir.dt.bfloat16
F32 = mybir.dt.float32
F32R = mybir.dt.float32r
NP_BF16 = ml_dtypes.bfloat16

P = 128
B, T, C = 4, 2048, 1024
H = 16
D = 64
HL = 8            # heads per core
NPAIR = HL // 2   # head pairs per core
NL = HL * D       # 512: local qkv width
CT = C // P       # 8 contraction tiles over C
DT = NL // P      # 4 contraction tiles over local head dims
NTO = C // P      # 8 output tiles for proj
TCH = T // 512    # 4 t-chunks
NJT = T // P      # 16 j tiles


def _split_excess_waits(nc, limit=1):
    """This walrus build supports a single sem-wait per instruction; move
    excess waits emitted by Tile onto preceding same-engine NoOps."""
    n = 0
    for bb in nc.main_func.blocks:
        out = []
        changed = False
        for inst in bb.instructions:
            si = inst.sync_info
            if si is not None and len(si.on_wait) > limit:
                waits = list(si.on_wait)
                excess, keep = waits[:-limit], waits[-limit:]
                for i in range(0, len(excess), limit):
                    out.append(
                        mybir.InstNoOp(
                            name=f"waitsplit_{n}",
                            ins=[],
                            outs=[],
                            engine=inst.engine,
                            sync_info=mybir.SyncInfo(
                                on_wait=excess[i : i + limit], on_update=[]
                            ),
                        )
                    )
                    n += 1
                si.on_wait = keep
                changed = True
            out.append(inst)
        if changed:
            bb.instructions = out
    return n


def build_nc(split_waits=True):
    nc = bass.Bass()
    AF = mybir.ActivationFunctionType

    xT = nc.dram_tensor("xT", [P, TCH, CT, 512], BF16, kind="ExternalInput")
    wq = nc.dram_tensor("wq", [P, CT, NL], BF16, kind="ExternalInput")
    wk = nc.dram_tensor("wk", [P, CT, NL], BF16, kind="ExternalInput")
    wv = nc.dram_tensor("wv", [P, CT, NL], BF16, kind="ExternalInput")
    wp = nc.dram_tensor("wp", [P, DT, C], BF16, kind="ExternalInput")
    bq = nc.dram_tensor("bq", [P, NPAIR], F32, kind="ExternalInput")
    bk = nc.dram_tensor("bk", [P, NPAIR], F32, kind="ExternalInput")
    bv = nc.dram_tensor("bv", [P, NL], F32, kind="ExternalInput")
    bp = nc.dram_tensor("bp", [P, NTO], F32, kind="ExternalInput")
    msk = nc.dram_tensor("msk", [P, P], F32, kind="ExternalInput")
    outT = nc.dram_tensor("outT", [P, NTO, T], BF16, kind="ExternalOutput")

    with tile.TileContext(nc) as tc, ExitStack() as ctx:
        persist = ctx.enter_context(tc.tile_pool(name="persist", bufs=1))
        # PSUM budget (8 banks): s: [128,2,512] = 2 banks x 2 bufs = 4;
        # y: [65,2,512] = 2 banks; lb: [64,2,512] = 2 banks.
        spsum = ctx.enter_context(tc.tile_pool(name="spsum", bufs=2, space="PSUM"))
        ypsum = ctx.enter_context(tc.tile_pool(name="ypsum", bufs=1, space="PSUM"))
        lpsum = ctx.enter_context(tc.tile_pool(name="lpsum", bufs=1, space="PSUM"))
        work = ctx.enter_context(tc.tile_pool(name="work", bufs=3))

        # ---- persistent SBUF tensors ----
        qT = persist.tile([P, NPAIR, T], BF16)   # [2x64d, pair, t]
        kT = persist.tile([P, NPAIR, T], BF16)
        vA = persist.tile([P, NJT, HL, D + 1], BF16)  # [j, jt, head, d|ones]
        yU = persist.tile([P, DT, T], BF16)  # y.T pair-packed; normalized in place
        onesP = persist.tile([P, D], BF16)   # lhsT rows for PE partition-broadcast
        # l rows spread to partition 32*pr (per head) for batched reciprocal
        lrowA = persist.tile([P, TCH, 512], BF16)
        lrowB = persist.tile([P, TCH, 512], BF16)
        linvA = persist.tile([P, TCH, 512], BF16)
        linvB = persist.tile([P, TCH, 512], BF16)
        xs = persist.tile([P, TCH, CT, 512], BF16)
        wqs = persist.tile([P, CT, NL], BF16)
        wks = persist.tile([P, CT, NL], BF16)
        wvs = persist.tile([P, CT, NL], BF16)
        wps = persist.tile([P, DT, C], BF16)
        bqs = persist.tile([P, NPAIR], F32)
        bks = persist.tile([P, NPAIR], F32)
        bvs = persist.tile([P, NL], F32)
        bps = persist.tile([P, NTO], F32)
        msks = persist.tile([P, 1, P], F32)

        # ---- all input DMAs up front, spread across 3 rings ----
        # gpsimd ring: weights/biases (cheap issue, Pool engine is idle)
        nc.gpsimd.dma_start(wvs[:], wv[:])
        nc.gpsimd.dma_start(wqs[:], wq[:])
        nc.gpsimd.dma_start(wks[:], wk[:])
        nc.gpsimd.dma_start(bqs[:], bq[:])
        nc.gpsimd.dma_start(bks[:], bk[:])
        nc.gpsimd.dma_start(bvs[:], bv[:])
        nc.gpsimd.dma_start(msks[:, 0, :], msk[:])
        nc.gpsimd.dma_start(wps[:], wp[:])
        nc.gpsimd.dma_start(bps[:], bp[:])
        # x chunks: halves on sync + scalar rings, chunk order
        for tc_i in range(TCH):
            nc.sync.dma_start(xs[:, tc_i, 0:4, :], xT[:, tc_i, 0:4, :])
            nc.scalar.dma_start(xs[:, tc_i, 4:8, :], xT[:, tc_i, 4:8, :])

        nc.vector.memset(vA[:, :, :, D : D + 1], 1.0)
        nc.vector.memset(onesP[:], 1.0)

        # ---------------- emitters ----------------
        def emit_v(tt):
            ps = spsum.tile([P, 2, 512], F32, tag="s")
            for ct in range(CT):
                nc.tensor.matmul(
                    ps[:, 0, :],
                    lhsT=xs[:, tt // 4, ct, (tt % 4) * P : (tt % 4 + 1) * P],
                    rhs=wvs[:, ct, :],
                    start=(ct == 0),
                    stop=(ct == CT - 1),
                )
            nc.vector.tensor_tensor(
                out=vA[:, tt, :, 0:D],
                in0=ps[:, 0, :].rearrange("p (h d) -> p h d", h=HL),
                in1=bvs.rearrange("p (h d) -> p h d", h=HL),
                op=mybir.AluOpType.add,
            )

        def emit_qk(nt, tc_i):
            ps = spsum.tile([P, 2, 512], F32, tag="s")
            t_sl = slice(tc_i * 512, (tc_i + 1) * 512)
            for ct in range(CT):
                nc.tensor.matmul(
                    ps[:, 0, :],
                    lhsT=wqs[:, ct, nt * P : (nt + 1) * P],
                    rhs=xs[:, tc_i, ct, :],
                    start=(ct == 0),
                    stop=(ct == CT - 1),
                )
            for ct in range(CT):
                nc.tensor.matmul(
                    ps[:, 1, :],
                    lhsT=wks[:, ct, nt * P : (nt + 1) * P],
                    rhs=xs[:, tc_i, ct, :],
                    start=(ct == 0),
                    stop=(ct == CT - 1),
                )
            nc.vector.tensor_scalar(
                out=qT[:, nt, t_sl], in0=ps[:, 0, :],
                scalar1=bqs[:, nt : nt + 1], scalar2=None,
                op0=mybir.AluOpType.add,
            )
            nc.vector.tensor_scalar(
                out=kT[:, nt, t_sl], in0=ps[:, 1, :],
                scalar1=bks[:, nt : nt + 1], scalar2=None,
                op0=mybir.AluOpType.add,
            )

        n_out_dma = [0]

        def emit_proj(nt, tc_i):
            t_sl = slice(tc_i * 512, (tc_i + 1) * 512)
            ps = spsum.tile([P, 2, 512], F32, tag="s")
            for dt in range(DT):
                nc.tensor.matmul(
                    ps[:, 0, :],
                    lhsT=wps[:, dt, nt * P : (nt + 1) * P],
                    rhs=yU[:, dt, t_sl],
                    start=(dt == 0),
                    stop=(dt == DT - 1),
                )
            ot = work.tile([P, 512], BF16, tag="o")
            nc.vector.tensor_scalar(
                out=ot[:], in0=ps[:, 0, :],
                scalar1=bps[:, nt : nt + 1], scalar2=None,
                op0=mybir.AluOpType.add,
            )
            eng = nc.gpsimd if n_out_dma[0] % 2 == 0 else nc.sync
            n_out_dma[0] += 1
            eng.dma_start(outT[:, nt, t_sl], ot[:])

        # ---- filler pump: units of (n_matmuls, emit_fn) consumed between
        # attention steps to keep the PE stream dense. Debt-carrying so big
        # units pop at the right average rate. ----
        filler = deque()
        debt = [0.0]

        def pump(budget):
            debt[0] += budget
            while filler and debt[0] >= filler[0][0]:
                n, fn = filler.popleft()
                fn()
                debt[0] -= n

        def flush():
            while filler:
                n, fn = filler.popleft()
                fn()
            debt[0] = 0.0

        def attention(pr, ic, pending_norms, budget):
            """Attention for head pair pr on i-chunk ic. pending_norms is a
            list of the previous chunk's deferred (PE broadcast + DVE
            multiply) closures, drained one per jt step."""
            hA, hB = 2 * pr, 2 * pr + 1
            njt = 4 * ic + 4
            i0 = ic * 512
            y = ypsum.tile([D + 1, 2, 512], F32, tag="y")
            sts = {}

            def emit_scores(jt):
                st = spsum.tile([P, 2, 512], F32, tag="s")
                sts[jt] = st
                ow = max(0, jt * P - i0)
                j_sl = slice(jt * P, (jt + 1) * P)
                i_sl = slice(i0 + ow, i0 + 512)
                nc.tensor.matmul(
                    st[:, 0, ow:512],
                    lhsT=kT[0:D, pr, j_sl],
                    rhs=qT[0:D, pr, i_sl],
                    start=True, stop=True,
                    tile_position=(0, 0),
                )
                nc.tensor.matmul(
                    st[:, 1, ow:512],
                    lhsT=kT[D:P, pr, j_sl],
                    rhs=qT[D:P, pr, i_sl],
                    start=True, stop=True,
                    tile_position=(64, 0),
                )
                if jt >= 4 * ic:  # diagonal tile: add -1e30 above diag
                    nc.vector.tensor_tensor(
                        out=st[:, :, ow : ow + P],
                        in0=st[:, :, ow : ow + P],
                        in1=msks[:].to_broadcast([P, 2, P]),
                        op=mybir.AluOpType.add,
                    )

            emit_scores(0)
            if njt > 1:
                emit_scores(1)
            for jt in range(njt):
                st = sts.pop(jt)
                ow = max(0, jt * P - i0)
                pt = work.tile([P, 2, 512], BF16, tag="p")
                nc.scalar.activation(
                    pt[:, :, ow:512], st[:, :, ow:512], AF.Exp, scale=0.125
                )
                if jt + 2 < njt:
                    emit_scores(jt + 2)
                nc.tensor.matmul(
                    y[:, 0, ow:512],
                    lhsT=vA[:, jt, hA, :],
                    rhs=pt[:, 0, ow:512],
                    start=(jt == 0),
                    stop=(jt == njt - 1),
                )
                nc.tensor.matmul(
                    y[:, 1, ow:512],
                    lhsT=vA[:, jt, hB, :],
                    rhs=pt[:, 1, ow:512],
                    start=(jt == 0),
                    stop=(jt == njt - 1),
                )
                if jt >= 1 and pending_norms:
                    pending_norms.pop(0)()
                pump(budget)

            # ---- per-pair epilogue: unnormalized y out of PSUM, l rows
            # staged bf16 and spread (via idle gpsimd DMAs) onto partition
            # 32*pr for the per-chunk batched reciprocal ----
            i_sl = slice(i0, i0 + 512)
            nc.vector.tensor_copy(yU[0:D, pr, i_sl], y[0:D, 0, :])
            nc.vector.tensor_copy(yU[D:P, pr, i_sl], y[0:D, 1, :])
            lcp = work.tile([1, 2, 512], BF16, tag="lv")
            nc.vector.tensor_copy(lcp[:], y[D : D + 1, :, :])
            r = 32 * pr
            nc.gpsimd.dma_start(lrowA[r : r + 1, ic, :], lcp[0:1, 0, :])
            nc.gpsimd.dma_start(lrowB[r : r + 1, ic, :], lcp[0:1, 1, :])

        def chunk_norm(ic):
            """Emit the batched 1/l for chunk ic now (DVE); return a closure
            with the PE broadcasts + DVE multiplies to run a bit later."""
            with nc.allow_low_precision("softmax denom in bf16 is plenty"):
                nc.vector.reciprocal(linvA[0:97, ic, :], lrowA[0:97, ic, :])
                nc.vector.reciprocal(linvB[0:97, ic, :], lrowB[0:97, ic, :])
            i_sl = slice(ic * 512, (ic + 1) * 512)

            def norm_mm(pr):
                r = 32 * pr
                lb = lpsum.tile([D, 2, 512], F32, tag="lb")
                nc.tensor.matmul(
                    lb[:, 0, :], lhsT=onesP[r : r + 1, :],
                    rhs=linvA[r : r + 1, ic, :], start=True, stop=True,
                    tile_position=(r, 0),
                )
                nc.tensor.matmul(
                    lb[:, 1, :], lhsT=onesP[r : r + 1, :],
                    rhs=linvB[r : r + 1, ic, :], start=True, stop=True,
                    tile_position=(r, 0),
                )
                nc.vector.tensor_tensor(
                    out=yU[0:D, pr, i_sl], in0=yU[0:D, pr, i_sl],
                    in1=lb[:, 0, :], op=mybir.AluOpType.mult,
                )
                nc.vector.tensor_tensor(
                    out=yU[D:P, pr, i_sl], in0=yU[D:P, pr, i_sl],
                    in1=lb[:, 1, :], op=mybir.AluOpType.mult,
                )

            return [(lambda p=pr: norm_mm(p)) for pr in range(NPAIR)]

        # ---------------- program ----------------
        # prologue: chunk-0 v and pair-0 q/k; remaining chunk-0 q/k pairs are
        # emitted inline right after each pair's attention (they are the next
        # pair's hard dependency; ic0 is PE-bound anyway)
        for tt in range(4):
            emit_v(tt)
        emit_qk(0, 0)

        pending = []
        for ic in range(TCH):
            if ic + 1 < TCH:
                for tt in range(4 * (ic + 1), 4 * (ic + 1) + 4):
                    filler.append((8, (lambda t=tt: emit_v(t))))
                for pr in range(NPAIR):
                    filler.append((16, (lambda p=pr, c=ic + 1: emit_qk(p, c))))
            if ic == TCH - 1:
                # deferred proj for chunks 0..2 pumps under the final (ACT
                # bound) chunk's attention. Chunk-2 units are appended last so
                # they pop only after chunk-2's pending norm has been emitted
                # (it runs at jt 1-4, the c2 units pop from jt ~26).
                for c in range(TCH - 1):
                    for nt in range(NTO):
                        filler.append((4, (lambda n=nt, cc=c: emit_proj(n, cc))))
            njts = NPAIR * (4 * ic + 4)
            budget = sum(n for n, _ in filler) / njts + 1.0
            for pr in range(NPAIR):
                attention(pr, ic, pending if pr == 0 else [], budget)
                if ic == 0 and pr + 1 < NPAIR:
                    emit_qk(pr + 1, 0)
            pending = chunk_norm(ic)
            # drain any leftover fillers before moving to the next chunk's
            # attention (they are that chunk's dependencies)
            flush()

        for fn in pending:
            fn()
        for nt in range(NTO):
            emit_proj(nt, TCH - 1)

    if split_waits:
        _split_excess_waits(nc, 1)
    return nc


def shard_inputs(x, w_attn, b_attn, w_proj, b_proj):
    """Build the 8 per-core input dicts (core = 2*batch + head_group)."""
    x = np.asarray(x, dtype=np.float32)
    w_attn = np.asarray(w_attn, dtype=np.float32)
    b_attn = np.asarray(b_attn, dtype=np.float32)
    w_proj = np.asarray(w_proj, dtype=np.float32)
    b_proj = np.asarray(b_proj, dtype=np.float32)

    # additive causal mask for a diagonal 128x128 block of S.T ([j, i]):
    # 0 where j <= i, -1e30 above the diagonal.
    pp = np.arange(P)
    msk = np.where(pp[:, None] <= pp[None, :], 0.0, -1e30).astype(np.float32)

    def wtile(w2d, ncols):  # [C_rows, ncols] -> [P, rows//P, ncols] bf16
        r = w2d.shape[0]
        return np.ascontiguousarray(
            w2d.reshape(r // P, P, ncols).transpose(1, 0, 2)
        ).astype(NP_BF16)

    in_maps = []
    for core in range(8):
        b, hg = divmod(core, 2)
        q0 = hg * NL
        xt = np.ascontiguousarray(x[b].T)  # [C, T]
        m = {
            "xT": np.ascontiguousarray(
                xt.reshape(CT, P, TCH, 512).transpose(1, 2, 0, 3)
            ).astype(NP_BF16),
            "wq": wtile(w_attn[:, q0 : q0 + NL], NL),
            "wk": wtile(w_attn[:, C + q0 : C + q0 + NL], NL),
            "wv": wtile(w_attn[:, 2 * C + q0 : 2 * C + q0 + NL], NL),
            "wp": wtile(w_proj[q0 : q0 + NL, :], C),
            "bq": np.ascontiguousarray(
                b_attn[q0 : q0 + NL].reshape(NPAIR, P).T
            ).astype(np.float32),
            "bk": np.ascontiguousarray(
                b_attn[C + q0 : C + q0 + NL].reshape(NPAIR, P).T
            ).astype(np.float32),
            "bv": np.broadcast_to(
                b_attn[2 * C + q0 : 2 * C + q0 + NL], (P, NL)
            ).astype(np.float32),
            "bp": (
                np.ascontiguousarray(b_proj.reshape(NTO, P).T).astype(np.float32)
                if hg == 0
                else np.zeros((P, NTO), np.float32)
            ),
            "msk": msk,
        }
        in_maps.append(m)
    return in_maps


def unshard_output(results):
    """Combine 8 per-core outT [P, NTO, T] bf16 partials into [B, T, C] fp32."""
    out = np.empty((B, T, C), dtype=np.float32)
    for b in range(B):
        acc = results[2 * b]["outT"].astype(np.float32) + results[
            2 * b + 1
        ]["outT"].astype(np.float32)
        # [P, NTO, T] -> [C, T] -> [T, C]
        out[b] = acc.transpose(1, 0, 2).reshape(C, T).T
    return out


_NC_CACHE = {}


def kernel(x, w_attn, b_attn, w_proj, b_proj):
    if "nc" not in _NC_CACHE:
        _NC_CACHE["nc"] = build_nc()
    nc = _NC_CACHE["nc"]
    in_maps = shard_inputs(x, w_attn, b_attn, w_proj, b_proj)
    res = run_bass_kernel_spmd(nc, in_maps, core_ids=list(range(8)))
    return unshard_output(res.results)


# revision 27
# speedup vs baseline: 1.0048x; 1.0048x over previous
"""Causal self-attention kernel for 8 Trainium2 NeuronCores.

Problem: B=4, T=2048, C=1024, H=16 heads (D=64).
Sharding: data-parallel over batch (4) x tensor-parallel over heads (2 groups
of 8 heads). Core c handles batch c//2, head-group c%2. Each core computes
qkv for its 8 heads, full causal attention on TxT scores, and its partial
projection output; the host sums the two head-group partials per batch.

v2 design notes (vs the v1 baseline at 382us):
  - chunk-outer pipeline: for each 512-wide i-chunk, all 4 head pairs run
    attention back to back while "filler" matmuls (next chunk's v/q/k, and
    deferred proj tiles) are pumped into the PE stream between attention
    steps. This keeps the tensor engine's HAM clock-gate warm (2.4 GHz needs
    >3.4us of continuous PE activity) and hides QKV/proj almost entirely
    under the exp() stream on the scalar engine, which is the hard floor
    (~150us of exp work per core).
  - all input DMAs are issued up front across three queues (gpsimd for
    weights, sync+scalar for the 8 x-halves) so compute never starves.
  - softmax denominator: v carries a ones column (PSUM row 64 = l partial);
    1/l via reciprocal_approx_fast straight off PSUM (no DMA lane-spread
    round trips), PE ones-column broadcast, in-place DVE multiply.
  - scores for a pair are computed in one [128,2,512] PSUM tile via the
    64-row tile_position trick (two matmuls co-execute).
  - proj outputs staged bf16 and DMA'd on gpsimd/sync rings; host sums the
    two head-group partials in fp32.
"""

import sys

if "/opt/trn_rl_repo" not in sys.path:
    sys.path.insert(0, "/opt/trn_rl_repo")

from collections import deque
from contextlib import ExitStack

import ml_dtypes
import numpy as np

import concourse.bass as bass
import concourse.mybir as mybir
import concourse.tile as tile
from concourse.bass_utils import run_bass_kernel_spmd

BF16 = mybir.dt.bfloat16
F32 = mybir.dt.float32
F32R = mybir.dt.float32r
NP_BF16 = ml_dtypes.bfloat16

P = 128
B, T, C = 4, 2048, 1024
H = 16
D = 64
HL = 8            # heads per core
NPAIR = HL // 2   # head pairs per core
NL = HL * D       # 512: local qkv width
CT = C // P       # 8 contraction tiles over C
DT = NL // P      # 4 contraction tiles over local head dims
NTO = C // P      # 8 output tiles for proj
TCH = T // 512    # 4 t-chunks
NJT = T // P      # 16 j tiles


def _split_excess_waits(nc, limit=1):
    """This walrus build supports a single sem-wait per instruction; move
    excess waits emitted by Tile onto preceding same-engine NoOps."""
    n = 0
    for bb in nc.main_func.blocks:
        out = []
        changed = False
        for inst in bb.instructions:
            si = inst.sync_info
            if si is not None and len(si.on_wait) > limit:
                waits = list(si.on_wait)
                excess, keep = waits[:-limit], waits[-limit:]
                for i in range(0, len(excess), limit):
                    out.append(
                        mybir.InstNoOp(
                            name=f"waitsplit_{n}",
                            ins=[],
                            outs=[],
                            engine=inst.engine,
                            sync_info=mybir.SyncInfo(
                                on_wait=excess[i : i + limit], on_update=[]
                            ),
                        )
                    )
                    n += 1
                si.on_wait = keep
                changed = True
            out.append(inst)
        if changed:
            bb.instructions = out
    return n


def build_nc(split_waits=True):
    nc = bass.Bass()
    AF = mybir.ActivationFunctionType

    xT = nc.dram_tensor("xT", [P, TCH, CT, 512], BF16, kind="ExternalInput")
    wq = nc.dram_tensor("wq", [P, CT, NL], BF16, kind="ExternalInput")
    wk = nc.dram_tensor("wk", [P, CT, NL], BF16, kind="ExternalInput")
    wv = nc.dram_tensor("wv", [P, CT, NL], BF16, kind="ExternalInput")
    wp = nc.dram_tensor("wp", [P, DT, C], BF16, kind="ExternalInput")
    bq = nc.dram_tensor("bq", [P, NPAIR], F32, kind="ExternalInput")
    bk = nc.dram_tensor("bk", [P, NPAIR], F32, kind="ExternalInput")
    bv = nc.dram_tensor("bv", [P, NL], F32, kind="ExternalInput")
    bp = nc.dram_tensor("bp", [P, NTO], F32, kind="ExternalInput")
    msk = nc.dram_tensor("msk", [P, P], F32, kind="ExternalInput")
    outT = nc.dram_tensor("outT", [P, NTO, T], BF16, kind="ExternalOutput")

    with tile.TileContext(nc) as tc, ExitStack() as ctx:
        persist = ctx.enter_context(tc.tile_pool(name="persist", bufs=1))
        # PSUM budget (8 banks): s: [128,2,512] = 2 banks x 2 bufs = 4;
        # y: [65,2,512] = 2 banks; lb: [64,2,512] = 2 banks.
        spsum = ctx.enter_context(tc.tile_pool(name="spsum", bufs=2, space="PSUM"))
        ypsum = ctx.enter_context(tc.tile_pool(name="ypsum", bufs=1, space="PSUM"))
        lpsum = ctx.enter_context(tc.tile_pool(name="lpsum", bufs=1, space="PSUM"))
        work = ctx.enter_context(tc.tile_pool(name="work", bufs=3))

        # ---- persistent SBUF tensors ----
        qT = persist.tile([P, NPAIR, T], BF16)   # [2x64d, pair, t]
        kT = persist.tile([P, NPAIR, T], BF16)
        vA = persist.tile([P, NJT, HL, D + 1], BF16)  # [j, jt, head, d|ones]
        yU = persist.tile([P, DT, T], BF16)  # y.T pair-packed; normalized in place
        onesP = persist.tile([P, D], BF16)   # lhsT rows for PE partition-broadcast
        # l rows spread to partition 32*pr (per head) for batched reciprocal
        lrowA = persist.tile([P, TCH, 512], BF16)
        lrowB = persist.tile([P, TCH, 512], BF16)
        linvA = persist.tile([P, TCH, 512], BF16)
        linvB = persist.tile([P, TCH, 512], BF16)
        xs = persist.tile([P, TCH, CT, 512], BF16)
        wqs = persist.tile([P, CT, NL], BF16)
        wks = persist.tile([P, CT, NL], BF16)
        wvs = persist.tile([P, CT, NL], BF16)
        wps = persist.tile([P, DT, C], BF16)
        bqs = persist.tile([P, NPAIR], F32)
        bks = persist.tile([P, NPAIR], F32)
        bvs = persist.tile([P, NL], F32)
        bps = persist.tile([P, NTO], F32)
        msks = persist.tile([P, 1, P], F32)

        nc.vector.memset(vA[:, :, :, D : D + 1], 1.0)
        nc.vector.memset(onesP[:], 1.0)

        # ---- all input DMAs up front, spread across 4 rings ----
        # gpsimd ring: weights/biases only (kept clear for the l-spread DMAs)
        nc.gpsimd.dma_start(wvs[:], wv[:])
        nc.gpsimd.dma_start(wqs[:], wq[:])
        nc.gpsimd.dma_start(wks[:], wk[:])
        nc.gpsimd.dma_start(bqs[:], bq[:])
        nc.gpsimd.dma_start(bks[:], bk[:])
        nc.gpsimd.dma_start(bvs[:], bv[:])
        nc.gpsimd.dma_start(msks[:, 0, :], msk[:])
        nc.gpsimd.dma_start(wps[:], wp[:])
        nc.gpsimd.dma_start(bps[:], bp[:])
        # x chunks: halves on sync + scalar rings, chunk order
        for tc_i in range(TCH):
            nc.sync.dma_start(xs[:, tc_i, 0:4, :], xT[:, tc_i, 0:4, :])
            nc.scalar.dma_start(xs[:, tc_i, 4:8, :], xT[:, tc_i, 4:8, :])

        # ---- PE warm-up: ~5us of garbage matmuls while the x DMA lands,
        # so the HAM clock-gate is at 2.4 GHz when real work starts ----
        for w in range(12):
            ps = spsum.tile([P, 2, 512], F32, tag="s")
            nc.tensor.matmul(
                ps[0:D, 0, :], lhsT=onesP[:], rhs=lrowA[:, 0, :],
                start=True, stop=True,
            )

        # ---------------- emitters ----------------
        def emit_v(tt):
            ps = spsum.tile([P, 2, 512], F32, tag="s")
            for ct in range(CT):
                nc.tensor.matmul(
                    ps[:, 0, :],
                    lhsT=xs[:, tt // 4, ct, (tt % 4) * P : (tt % 4 + 1) * P],
                    rhs=wvs[:, ct, :],
                    start=(ct == 0),
                    stop=(ct == CT - 1),
                )
            nc.vector.tensor_tensor(
                out=vA[:, tt, :, 0:D],
                in0=ps[:, 0, :].rearrange("p (h d) -> p h d", h=HL),
                in1=bvs.rearrange("p (h d) -> p h d", h=HL),
                op=mybir.AluOpType.add,
            )

        def emit_qk(nt, tc_i):
            ps = spsum.tile([P, 2, 512], F32, tag="s")
            t_sl = slice(tc_i * 512, (tc_i + 1) * 512)
            for ct in range(CT):
                nc.tensor.matmul(
                    ps[:, 0, :],
                    lhsT=wqs[:, ct, nt * P : (nt + 1) * P],
                    rhs=xs[:, tc_i, ct, :],
                    start=(ct == 0),
                    stop=(ct == CT - 1),
                )
            for ct in range(CT):
                nc.tensor.matmul(
                    ps[:, 1, :],
                    lhsT=wks[:, ct, nt * P : (nt + 1) * P],
                    rhs=xs[:, tc_i, ct, :],
                    start=(ct == 0),
                    stop=(ct == CT - 1),
                )
            nc.vector.tensor_scalar(
                out=qT[:, nt, t_sl], in0=ps[:, 0, :],
                scalar1=bqs[:, nt : nt + 1], scalar2=None,
                op0=mybir.AluOpType.add,
            )
            nc.vector.tensor_scalar(
                out=kT[:, nt, t_sl], in0=ps[:, 1, :],
                scalar1=bks[:, nt : nt + 1], scalar2=None,
                op0=mybir.AluOpType.add,
            )

        n_out_dma = [0]

        def emit_proj(nt, tc_i):
            t_sl = slice(tc_i * 512, (tc_i + 1) * 512)
            ps = spsum.tile([P, 2, 512], F32, tag="s")
            for dt in range(DT):
                nc.tensor.matmul(
                    ps[:, 0, :],
                    lhsT=wps[:, dt, nt * P : (nt + 1) * P],
                    rhs=yU[:, dt, t_sl],
                    start=(dt == 0),
                    stop=(dt == DT - 1),
                )
            ot = work.tile([P, 512], BF16, tag="o")
            nc.vector.tensor_scalar(
                out=ot[:], in0=ps[:, 0, :],
                scalar1=bps[:, nt : nt + 1], scalar2=None,
                op0=mybir.AluOpType.add,
            )
            # keep the gpsimd ring clear (it carries the l-spread DMAs);
            # mid-run outputs go on sync, the final chunk fans out over three
            # rings so the tail drains fast (ACT is idle by then)
            if tc_i < TCH - 1:
                eng = nc.sync
            else:
                eng = [nc.sync, nc.scalar][n_out_dma[0] % 2]
                n_out_dma[0] += 1
            eng.dma_start(outT[:, nt, t_sl], ot[:])

        # ---- filler pump: units of (n_matmuls, emit_fn) consumed between
        # attention steps to keep the PE stream dense. Debt-carrying so big
        # units pop at the right average rate. ----
        filler = deque()
        debt = [0.0]

        def pump(budget):
            debt[0] += budget
            while filler and debt[0] >= filler[0][0]:
                n, fn = filler.popleft()
                fn()
                debt[0] -= n

        def flush():
            while filler:
                n, fn = filler.popleft()
                fn()
            debt[0] = 0.0

        def attention(pr, ic, pending_norms, budget):
            """Attention for head pair pr on i-chunk ic. pending_norms is a
            list of the previous chunk's deferred (PE broadcast + DVE
            multiply) closures, drained one per jt step."""
            hA, hB = 2 * pr, 2 * pr + 1
            njt = 4 * ic + 4
            i0 = ic * 512
            y = ypsum.tile([D + 1, 2, 512], F32, tag="y")
            sts = {}

            def emit_scores(jt):
                st = spsum.tile([P, 2, 512], F32, tag="s")
                sts[jt] = st
                ow = max(0, jt * P - i0)
                j_sl = slice(jt * P, (jt + 1) * P)
                i_sl = slice(i0 + ow, i0 + 512)
                nc.tensor.matmul(
                    st[:, 0, ow:512],
                    lhsT=kT[0:D, pr, j_sl],
                    rhs=qT[0:D, pr, i_sl],
                    start=True, stop=True,
                    tile_position=(0, 0),
                )
                nc.tensor.matmul(
                    st[:, 1, ow:512],
                    lhsT=kT[D:P, pr, j_sl],
                    rhs=qT[D:P, pr, i_sl],
                    start=True, stop=True,
                    tile_position=(64, 0),
                )
                if jt >= 4 * ic:  # diagonal tile: add -1e30 above diag
                    nc.vector.tensor_tensor(
                        out=st[:, :, ow : ow + P],
                        in0=st[:, :, ow : ow + P],
                        in1=msks[:].to_broadcast([P, 2, P]),
                        op=mybir.AluOpType.add,
                    )

            emit_scores(0)
            if njt > 1:
                emit_scores(1)
            for jt in range(njt):
                st = sts.pop(jt)
                ow = max(0, jt * P - i0)
                pt = work.tile([P, 2, 512], BF16, tag="p")
                nc.scalar.activation(
                    pt[:, :, ow:512], st[:, :, ow:512], AF.Exp, scale=0.125
                )
                # PE work that does NOT depend on exp(jt) goes here, so the
                # in-order PE queue has filler to chew while exp(jt) runs
                # (scores(jt+2) reuses exp(jt)'s PSUM buffer, PV needs its
                # output — both stall the queue head otherwise)
                if jt >= 1 and pending_norms:
                    pending_norms.pop(0)()
                pump(budget)
                if jt + 2 < njt:
                    emit_scores(jt + 2)
                nc.tensor.matmul(
                    y[:, 0, ow:512],
                    lhsT=vA[:, jt, hA, :],
                    rhs=pt[:, 0, ow:512],
                    start=(jt == 0),
                    stop=(jt == njt - 1),
                )
                nc.tensor.matmul(
                    y[:, 1, ow:512],
                    lhsT=vA[:, jt, hB, :],
                    rhs=pt[:, 1, ow:512],
                    start=(jt == 0),
                    stop=(jt == njt - 1),
                )

            # ---- per-pair epilogue: unnormalized y out of PSUM, l rows
            # staged bf16 and spread (via idle gpsimd DMAs) onto partition
            # 32*pr for the per-chunk batched reciprocal ----
            i_sl = slice(i0, i0 + 512)
            nc.vector.tensor_copy(yU[0:D, pr, i_sl], y[0:D, 0, :])
            nc.vector.tensor_copy(yU[D:P, pr, i_sl], y[0:D, 1, :])
            lcp = work.tile([1, 2, 512], BF16, tag="lv")
            nc.vector.tensor_copy(lcp[:], y[D : D + 1, :, :])
            r = 32 * pr
            nc.gpsimd.dma_start(lrowA[r : r + 1, ic, :], lcp[0:1, 0, :])
            nc.gpsimd.dma_start(lrowB[r : r + 1, ic, :], lcp[0:1, 1, :])

        def chunk_norm(ic):
            """Emit the batched 1/l for chunk ic now (DVE); return a closure
            with the PE broadcasts + DVE multiplies to run a bit later."""
            with nc.allow_low_precision("softmax denom in bf16 is plenty"):
                nc.vector.reciprocal(linvA[0:97, ic, :], lrowA[0:97, ic, :])
                nc.vector.reciprocal(linvB[0:97, ic, :], lrowB[0:97, ic, :])
            i_sl = slice(ic * 512, (ic + 1) * 512)

            def norm_mm(pr):
                r = 32 * pr
                lb = lpsum.tile([D, 2, 512], F32, tag="lb")
                nc.tensor.matmul(
                    lb[:, 0, :], lhsT=onesP[r : r + 1, :],
                    rhs=linvA[r : r + 1, ic, :], start=True, stop=True,
                    tile_position=(r, 0),
                )
                nc.tensor.matmul(
                    lb[:, 1, :], lhsT=onesP[r : r + 1, :],
                    rhs=linvB[r : r + 1, ic, :], start=True, stop=True,
                    tile_position=(r, 0),
                )
                nc.vector.tensor_tensor(
                    out=yU[0:D, pr, i_sl], in0=yU[0:D, pr, i_sl],
                    in1=lb[:, 0, :], op=mybir.AluOpType.mult,
                )
                nc.vector.tensor_tensor(
                    out=yU[D:P, pr, i_sl], in0=yU[D:P, pr, i_sl],
                    in1=lb[:, 1, :], op=mybir.AluOpType.mult,
                )

            return [(lambda p=pr: norm_mm(p)) for pr in range(NPAIR)]

        # ---------------- program ----------------
        # prologue: chunk-0 v and pair-0 q/k; remaining chunk-0 q/k pairs are
        # emitted inline right after each pair's attention (they are the next
        # pair's hard dependency; ic0 is PE-bound anyway)
        for tt in range(4):
            emit_v(tt)
        emit_qk(0, 0)

        pending = []
        for ic in range(TCH):
            if ic + 1 < TCH:
                for tt in range(4 * (ic + 1), 4 * (ic + 1) + 4):
                    filler.append((8, (lambda t=tt: emit_v(t))))
                for pr in range(NPAIR):
                    filler.append((16, (lambda p=pr, c=ic + 1: emit_qk(p, c))))
            if ic == TCH - 1:
                # deferred proj for chunks 0..2 pumps under the final (ACT
                # bound) chunk's attention. Chunk-2 units are appended last so
                # they pop only after chunk-2's pending norm has been emitted
                # (it runs at jt 1-4, the c2 units pop from jt ~26).
                for c in range(TCH - 1):
                    for nt in range(NTO):
                        filler.append((4, (lambda n=nt, cc=c: emit_proj(n, cc))))
            njts = NPAIR * (4 * ic + 4)
            budget = sum(n for n, _ in filler) / njts + 1.0
            for pr in range(NPAIR):
                attention(pr, ic, pending if pr == 0 else [], budget)
                if ic == 0 and pr + 1 < NPAIR:
                    emit_qk(pr + 1, 0)
            pending = chunk_norm(ic)
            # drain any leftover fillers before moving to the next chunk's
            # attention (they are that chunk's dependencies)
            flush()

        for fn in pending:
            fn()
        for nt in range(NTO):
            emit_proj(nt, TCH - 1)

    if split_waits:
        _split_excess_waits(nc, 1)
    return nc


def shard_inputs(x, w_attn, b_attn, w_proj, b_proj):
    """Build the 8 per-core input dicts (core = 2*batch + head_group)."""
    x = np.asarray(x, dtype=np.float32)
    w_attn = np.asarray(w_attn, dtype=np.float32)
    b_attn = np.asarray(b_attn, dtype=np.float32)
    w_proj = np.asarray(w_proj, dtype=np.float32)
    b_proj = np.asarray(b_proj, dtype=np.float32)

    # additive causal mask for a diagonal 128x128 block of S.T ([j, i]):
    # 0 where j <= i, -1e30 above the diagonal.
    pp = np.arange(P)
    msk = np.where(pp[:, None] <= pp[None, :], 0.0, -1e30).astype(np.float32)

    def wtile(w2d, ncols):  # [C_rows, ncols] -> [P, rows//P, ncols] bf16
        r = w2d.shape[0]
        return np.ascontiguousarray(
            w2d.reshape(r // P, P, ncols).transpose(1, 0, 2)
        ).astype(NP_BF16)

    in_maps = []
    for core in range(8):
        b, hg = divmod(core, 2)
        q0 = hg * NL
        xt = np.ascontiguousarray(x[b].T)  # [C, T]
        m = {
            "xT": np.ascontiguousarray(
                xt.reshape(CT, P, TCH, 512).transpose(1, 2, 0, 3)
            ).astype(NP_BF16),
            "wq": wtile(w_attn[:, q0 : q0 + NL], NL),
            "wk": wtile(w_attn[:, C + q0 : C + q0 + NL], NL),
            "wv": wtile(w_attn[:, 2 * C + q0 : 2 * C + q0 + NL], NL),
            "wp": wtile(w_proj[q0 : q0 + NL, :], C),
            "bq": np.ascontiguousarray(
                b_attn[q0 : q0 + NL].reshape(NPAIR, P).T
            ).astype(np.float32),
            "bk": np.ascontiguousarray(
                b_attn[C + q0 : C + q0 + NL].reshape(NPAIR, P).T
            ).astype(np.float32),
            "bv": np.broadcast_to(
                b_attn[2 * C + q0 : 2 * C + q0 + NL], (P, NL)
            ).astype(np.float32),
            "bp": (
                np.ascontiguousarray(b_proj.reshape(NTO, P).T).astype(np.float32)
                if hg == 0
                else np.zeros((P, NTO), np.float32)
            ),
            "msk": msk,
        }
        in_maps.append(m)
    return in_maps


def unshard_output(results):
    """Combine 8 per-core outT [P, NTO, T] bf16 partials into [B, T, C] fp32."""
    out = np.empty((B, T, C), dtype=np.float32)
    for b in range(B):
        acc = results[2 * b]["outT"].astype(np.float32) + results[
            2 * b + 1
        ]["outT"].astype(np.float32)
        # [P, NTO, T] -> [C, T] -> [T, C]
        out[b] = acc.transpose(1, 0, 2).reshape(C, T).T
    return out


_NC_CACHE = {}


def kernel(x, w_attn, b_attn, w_proj, b_proj):
    if "nc" not in _NC_CACHE:
        _NC_CACHE["nc"] = build_nc()
    nc = _NC_CACHE["nc"]
    in_maps = shard_inputs(x, w_attn, b_attn, w_proj, b_proj)
    res = run_bass_kernel_spmd(nc, in_maps, core_ids=list(range(8)))
    return unshard_output(res.results)


# revision 31
# speedup vs baseline: 1.0454x; 1.0404x over previous
"""Causal self-attention kernel for 8 Trainium2 NeuronCores.

Problem: B=4, T=2048, C=1024, H=16 heads (D=64).
Sharding: data-parallel over batch (4) x tensor-parallel over heads (2 groups
of 8 heads). Core c handles batch c//2, head-group c%2. Each core computes
qkv for its 8 heads, full causal attention on TxT scores, and its partial
projection output; the host sums the two head-group partials per batch.

v2 design notes (vs the v1 baseline at 382us):
  - chunk-outer pipeline: for each 512-wide i-chunk, all 4 head pairs run
    attention back to back while "filler" matmuls (next chunk's v/q/k, and
    deferred proj tiles) are pumped into the PE stream between attention
    steps. This keeps the tensor engine's HAM clock-gate warm (2.4 GHz needs
    >3.4us of continuous PE activity) and hides QKV/proj almost entirely
    under the exp() stream on the scalar engine, which is the hard floor
    (~150us of exp work per core).
  - all input DMAs are issued up front across three queues (gpsimd for
    weights, sync+scalar for the 8 x-halves) so compute never starves.
  - softmax denominator: v carries a ones column (PSUM row 64 = l partial);
    1/l via reciprocal_approx_fast straight off PSUM (no DMA lane-spread
    round trips), PE ones-column broadcast, in-place DVE multiply.
  - scores for a pair are computed in one [128,2,512] PSUM tile via the
    64-row tile_position trick (two matmuls co-execute).
  - proj outputs staged bf16 and DMA'd on gpsimd/sync rings; host sums the
    two head-group partials in fp32.
"""

import sys

if "/opt/trn_rl_repo" not in sys.path:
    sys.path.insert(0, "/opt/trn_rl_repo")

from collections import deque
from contextlib import ExitStack

import ml_dtypes
import numpy as np

import concourse.bass as bass
import concourse.mybir as mybir
import concourse.tile as tile
from concourse.bass_utils import run_bass_kernel_spmd

BF16 = mybir.dt.bfloat16
F32 = mybir.dt.float32
F32R = mybir.dt.float32r
NP_BF16 = ml_dtypes.bfloat16

P = 128
B, T, C = 4, 2048, 1024
H = 16
D = 64
HL = 8            # heads per core
NPAIR = HL // 2   # head pairs per core
NL = HL * D       # 512: local qkv width
CT = C // P       # 8 contraction tiles over C
DT = NL // P      # 4 contraction tiles over local head dims
NTO = C // P      # 8 output tiles for proj
TCH = T // 512    # 4 t-chunks
NJT = T // P      # 16 j tiles


def _split_excess_waits(nc, limit=1):
    """This walrus build supports a single sem-wait per instruction; move
    excess waits emitted by Tile onto preceding same-engine NoOps."""
    n = 0
    for bb in nc.main_func.blocks:
        out = []
        changed = False
        for inst in bb.instructions:
            si = inst.sync_info
            if si is not None and len(si.on_wait) > limit:
                waits = list(si.on_wait)
                excess, keep = waits[:-limit], waits[-limit:]
                for i in range(0, len(excess), limit):
                    out.append(
                        mybir.InstNoOp(
                            name=f"waitsplit_{n}",
                            ins=[],
                            outs=[],
                            engine=inst.engine,
                            sync_info=mybir.SyncInfo(
                                on_wait=excess[i : i + limit], on_update=[]
                            ),
                        )
                    )
                    n += 1
                si.on_wait = keep
                changed = True
            out.append(inst)
        if changed:
            bb.instructions = out
    return n


def build_nc(split_waits=True):
    nc = bass.Bass()
    AF = mybir.ActivationFunctionType

    xT = nc.dram_tensor("xT", [P, TCH, CT, 512], BF16, kind="ExternalInput")
    wq = nc.dram_tensor("wq", [P, CT, NL], BF16, kind="ExternalInput")
    wk = nc.dram_tensor("wk", [P, CT, NL], BF16, kind="ExternalInput")
    wv = nc.dram_tensor("wv", [P, CT, NL], BF16, kind="ExternalInput")
    wp = nc.dram_tensor("wp", [P, DT, C], BF16, kind="ExternalInput")
    bq = nc.dram_tensor("bq", [P, NPAIR], F32, kind="ExternalInput")
    bk = nc.dram_tensor("bk", [P, NPAIR], F32, kind="ExternalInput")
    bv = nc.dram_tensor("bv", [P, NL], F32, kind="ExternalInput")
    bp = nc.dram_tensor("bp", [P, NTO], F32, kind="ExternalInput")
    msk = nc.dram_tensor("msk", [P, P], F32, kind="ExternalInput")
    outT = nc.dram_tensor("outT", [P, NTO, T], BF16, kind="ExternalOutput")

    with tile.TileContext(nc) as tc, ExitStack() as ctx:
        persist = ctx.enter_context(tc.tile_pool(name="persist", bufs=1))
        # PSUM budget (8 banks): s: [128,2,512] = 2 banks x 2 bufs = 4;
        # y: [65,2,512] = 2 banks; lb: [64,2,512] = 2 banks.
        spsum = ctx.enter_context(tc.tile_pool(name="spsum", bufs=2, space="PSUM"))
        ypsum = ctx.enter_context(tc.tile_pool(name="ypsum", bufs=1, space="PSUM"))
        lpsum = ctx.enter_context(tc.tile_pool(name="lpsum", bufs=1, space="PSUM"))
        work = ctx.enter_context(tc.tile_pool(name="work", bufs=3))

        # ---- persistent SBUF tensors ----
        qT = persist.tile([P, NPAIR, T], BF16)   # [2x64d, pair, t]
        kT = persist.tile([P, NPAIR, T], BF16)
        vA = persist.tile([P, NJT, HL, D + 1], BF16)  # [j, jt, head, d|ones]
        yU = persist.tile([P, DT, T], BF16)  # y.T pair-packed; normalized in place
        onesP = persist.tile([P, D], BF16)   # lhsT rows for PE partition-broadcast
        # l rows spread to partition 32*pr (per head) for batched reciprocal
        lrowA = persist.tile([P, TCH, 512], BF16)
        lrowB = persist.tile([P, TCH, 512], BF16)
        linvA = persist.tile([P, TCH, 512], BF16)
        linvB = persist.tile([P, TCH, 512], BF16)
        xs = persist.tile([P, TCH, CT, 512], BF16)
        wqs = persist.tile([P, CT, NL], BF16)
        wks = persist.tile([P, CT, NL], BF16)
        wvs = persist.tile([P, CT, NL], BF16)
        wps = persist.tile([P, DT, C], BF16)
        bqs = persist.tile([P, NPAIR], F32)
        bks = persist.tile([P, NPAIR], F32)
        bvs = persist.tile([P, NL], F32)
        bps = persist.tile([P, NTO], F32)
        msks = persist.tile([P, 1, P], F32)

        nc.vector.memset(vA[:, :, :, D : D + 1], 1.0)
        nc.vector.memset(onesP[:], 1.0)

        # ---- all input DMAs up front, spread across 4 rings ----
        # gpsimd ring: weights/biases only (kept clear for the l-spread DMAs)
        nc.gpsimd.dma_start(wvs[:], wv[:])
        nc.gpsimd.dma_start(wqs[:], wq[:])
        nc.gpsimd.dma_start(wks[:], wk[:])
        nc.gpsimd.dma_start(bqs[:], bq[:])
        nc.gpsimd.dma_start(bks[:], bk[:])
        nc.gpsimd.dma_start(bvs[:], bv[:])
        nc.gpsimd.dma_start(msks[:, 0, :], msk[:])
        nc.gpsimd.dma_start(wps[:], wp[:])
        nc.gpsimd.dma_start(bps[:], bp[:])
        # x chunks: halves on sync + scalar rings, chunk order
        for tc_i in range(TCH):
            nc.sync.dma_start(xs[:, tc_i, 0:4, :], xT[:, tc_i, 0:4, :])
            nc.scalar.dma_start(xs[:, tc_i, 4:8, :], xT[:, tc_i, 4:8, :])

        # ---- PE warm-up: ~5us of garbage matmuls while the x DMA lands,
        # so the HAM clock-gate is at 2.4 GHz when real work starts ----
        for w in range(24):
            ps = spsum.tile([P, 2, 512], F32, tag="s")
            nc.tensor.matmul(
                ps[0:D, 0, :], lhsT=onesP[:], rhs=lrowA[:, 0, :],
                start=True, stop=True,
            )

        # ---------------- emitters ----------------
        def emit_v(tt):
            ps = spsum.tile([P, 2, 512], F32, tag="s")
            for ct in range(CT):
                nc.tensor.matmul(
                    ps[:, 0, :],
                    lhsT=xs[:, tt // 4, ct, (tt % 4) * P : (tt % 4 + 1) * P],
                    rhs=wvs[:, ct, :],
                    start=(ct == 0),
                    stop=(ct == CT - 1),
                )
            nc.vector.tensor_tensor(
                out=vA[:, tt, :, 0:D],
                in0=ps[:, 0, :].rearrange("p (h d) -> p h d", h=HL),
                in1=bvs.rearrange("p (h d) -> p h d", h=HL),
                op=mybir.AluOpType.add,
            )

        def emit_qk(nt, tc_i):
            ps = spsum.tile([P, 2, 512], F32, tag="s")
            t_sl = slice(tc_i * 512, (tc_i + 1) * 512)
            for ct in range(CT):
                nc.tensor.matmul(
                    ps[:, 0, :],
                    lhsT=wqs[:, ct, nt * P : (nt + 1) * P],
                    rhs=xs[:, tc_i, ct, :],
                    start=(ct == 0),
                    stop=(ct == CT - 1),
                )
            for ct in range(CT):
                nc.tensor.matmul(
                    ps[:, 1, :],
                    lhsT=wks[:, ct, nt * P : (nt + 1) * P],
                    rhs=xs[:, tc_i, ct, :],
                    start=(ct == 0),
                    stop=(ct == CT - 1),
                )
            nc.vector.tensor_scalar(
                out=qT[:, nt, t_sl], in0=ps[:, 0, :],
                scalar1=bqs[:, nt : nt + 1], scalar2=None,
                op0=mybir.AluOpType.add,
            )
            nc.vector.tensor_scalar(
                out=kT[:, nt, t_sl], in0=ps[:, 1, :],
                scalar1=bks[:, nt : nt + 1], scalar2=None,
                op0=mybir.AluOpType.add,
            )

        n_out_dma = [0]

        def emit_proj(nt, tc_i):
            t_sl = slice(tc_i * 512, (tc_i + 1) * 512)
            ps = spsum.tile([P, 2, 512], F32, tag="s")
            for dt in range(DT):
                nc.tensor.matmul(
                    ps[:, 0, :],
                    lhsT=wps[:, dt, nt * P : (nt + 1) * P],
                    rhs=yU[:, dt, t_sl],
                    start=(dt == 0),
                    stop=(dt == DT - 1),
                )
            ot = work.tile([P, 512], BF16, tag="o")
            nc.vector.tensor_scalar(
                out=ot[:], in0=ps[:, 0, :],
                scalar1=bps[:, nt : nt + 1], scalar2=None,
                op0=mybir.AluOpType.add,
            )
            # keep the gpsimd ring clear (it carries the l-spread DMAs);
            # mid-run outputs go on sync, the final chunk fans out over three
            # rings so the tail drains fast (ACT is idle by then)
            if tc_i < TCH - 1:
                eng = nc.sync
            else:
                eng = [nc.sync, nc.scalar][n_out_dma[0] % 2]
                n_out_dma[0] += 1
            eng.dma_start(outT[:, nt, t_sl], ot[:])

        # ---- filler pump: units of (n_matmuls, emit_fn) consumed between
        # attention steps to keep the PE stream dense. Debt-carrying so big
        # units pop at the right average rate. ----
        filler = deque()
        debt = [0.0]

        def pump(budget):
            debt[0] += budget
            while filler and debt[0] >= filler[0][0]:
                n, fn = filler.popleft()
                fn()
                debt[0] -= n

        def flush():
            while filler:
                n, fn = filler.popleft()
                fn()
            debt[0] = 0.0

        def attention(pr, ic, pending_norms, budget_fn):
            """Attention for head pair pr on i-chunk ic. pending_norms is a
            list of the previous chunk's deferred (PE broadcast + DVE
            multiply) closures, drained one per jt step once the chunk is a
            few steps in (their reciprocal input needs time to compute —
            popping them early blocks the in-order PE queue)."""
            hA, hB = 2 * pr, 2 * pr + 1
            njt = 4 * ic + 4
            i0 = ic * 512
            y = ypsum.tile([D + 1, 2, 512], F32, tag="y")
            sts = {}

            def emit_scores(jt):
                st = spsum.tile([P, 2, 512], F32, tag="s")
                sts[jt] = st
                ow = max(0, jt * P - i0)
                j_sl = slice(jt * P, (jt + 1) * P)
                i_sl = slice(i0 + ow, i0 + 512)
                nc.tensor.matmul(
                    st[:, 0, ow:512],
                    lhsT=kT[0:D, pr, j_sl],
                    rhs=qT[0:D, pr, i_sl],
                    start=True, stop=True,
                    tile_position=(0, 0),
                )
                nc.tensor.matmul(
                    st[:, 1, ow:512],
                    lhsT=kT[D:P, pr, j_sl],
                    rhs=qT[D:P, pr, i_sl],
                    start=True, stop=True,
                    tile_position=(64, 0),
                )
                if jt >= 4 * ic:  # diagonal tile: add -1e30 above diag
                    nc.vector.tensor_tensor(
                        out=st[:, :, ow : ow + P],
                        in0=st[:, :, ow : ow + P],
                        in1=msks[:].to_broadcast([P, 2, P]),
                        op=mybir.AluOpType.add,
                    )

            emit_scores(0)
            if njt > 1:
                emit_scores(1)
            for jt in range(njt):
                st = sts.pop(jt)
                ow = max(0, jt * P - i0)
                pt = work.tile([P, 2, 512], BF16, tag="p")
                nc.scalar.activation(
                    pt[:, :, ow:512], st[:, :, ow:512], AF.Exp, scale=0.125
                )
                # PE work that does NOT depend on exp(jt) goes here, so the
                # in-order PE queue has filler to chew while exp(jt) runs
                # (scores(jt+2) reuses exp(jt)'s PSUM buffer, PV needs its
                # output — both stall the queue head otherwise)
                if pending_norms and (pr > 0 or jt >= 5):
                    pending_norms.pop(0)()
                pump(budget_fn())
                if jt + 2 < njt:
                    emit_scores(jt + 2)
                nc.tensor.matmul(
                    y[:, 0, ow:512],
                    lhsT=vA[:, jt, hA, :],
                    rhs=pt[:, 0, ow:512],
                    start=(jt == 0),
                    stop=(jt == njt - 1),
                )
                nc.tensor.matmul(
                    y[:, 1, ow:512],
                    lhsT=vA[:, jt, hB, :],
                    rhs=pt[:, 1, ow:512],
                    start=(jt == 0),
                    stop=(jt == njt - 1),
                )

            # ---- per-pair epilogue: unnormalized y out of PSUM, l rows
            # staged bf16 and spread (via idle gpsimd DMAs) onto partition
            # 32*pr for the per-chunk batched reciprocal ----
            i_sl = slice(i0, i0 + 512)
            nc.vector.tensor_copy(yU[0:D, pr, i_sl], y[0:D, 0, :])
            nc.vector.tensor_copy(yU[D:P, pr, i_sl], y[0:D, 1, :])
            lcp = work.tile([1, 2, 512], BF16, tag="lv")
            nc.vector.tensor_copy(lcp[:], y[D : D + 1, :, :])
            r = 32 * pr
            nc.gpsimd.dma_start(lrowA[r : r + 1, ic, :], lcp[0:1, 0, :])
            nc.gpsimd.dma_start(lrowB[r : r + 1, ic, :], lcp[0:1, 1, :])

        def chunk_norm(ic):
            """Emit the batched 1/l for chunk ic now (DVE); return a closure
            with the PE broadcasts + DVE multiplies to run a bit later."""
            with nc.allow_low_precision("softmax denom in bf16 is plenty"):
                nc.vector.reciprocal(linvA[0:97, ic, :], lrowA[0:97, ic, :])
                nc.vector.reciprocal(linvB[0:97, ic, :], lrowB[0:97, ic, :])
            i_sl = slice(ic * 512, (ic + 1) * 512)

            def norm_mm(pr):
                r = 32 * pr
                lb = lpsum.tile([D, 2, 512], F32, tag="lb")
                nc.tensor.matmul(
                    lb[:, 0, :], lhsT=onesP[r : r + 1, :],
                    rhs=linvA[r : r + 1, ic, :], start=True, stop=True,
                    tile_position=(r, 0),
                )
                nc.tensor.matmul(
                    lb[:, 1, :], lhsT=onesP[r : r + 1, :],
                    rhs=linvB[r : r + 1, ic, :], start=True, stop=True,
                    tile_position=(r, 0),
                )
                nc.vector.tensor_tensor(
                    out=yU[0:D, pr, i_sl], in0=yU[0:D, pr, i_sl],
                    in1=lb[:, 0, :], op=mybir.AluOpType.mult,
                )
                nc.vector.tensor_tensor(
                    out=yU[D:P, pr, i_sl], in0=yU[D:P, pr, i_sl],
                    in1=lb[:, 1, :], op=mybir.AluOpType.mult,
                )

            return [(lambda p=pr: norm_mm(p)) for pr in range(NPAIR)]

        # ---------------- program ----------------
        # prologue: chunk-0 v and pair-0 q/k; remaining chunk-0 q/k pairs are
        # emitted inline right after each pair's attention (they are the next
        # pair's hard dependency; ic0 is PE-bound anyway)
        for tt in range(4):
            emit_v(tt)
        emit_qk(0, 0)

        pending = []
        for ic in range(TCH):
            if ic + 1 < TCH:
                for tt in range(4 * (ic + 1), 4 * (ic + 1) + 4):
                    filler.append((8, (lambda t=tt: emit_v(t))))
                for pr in range(NPAIR):
                    filler.append((16, (lambda p=pr, c=ic + 1: emit_qk(p, c))))
            if ic == TCH - 1:
                # deferred proj for chunks 0..2 pumps under the final (ACT
                # bound) chunk's attention. Chunk-2 units are appended last so
                # they pop only after chunk-2's pending norm has been emitted
                # (it runs at jt 1-4, the c2 units pop from jt ~26).
                for c in range(TCH - 1):
                    for nt in range(NTO):
                        filler.append((4, (lambda n=nt, cc=c: emit_proj(n, cc))))
            # dynamic pump budget: spread the remaining filler matmuls evenly
            # over the chunk's remaining jt steps (plus a small reserve that
            # keeps PE fed through the chunk-boundary reciprocal chain)
            njts = NPAIR * (4 * ic + 4)
            jt_ctr = [0]

            def budget_fn():
                jt_ctr[0] += 1
                left = sum(n for n, _ in filler)
                return left / max(njts + 6 - jt_ctr[0], 1)

            for pr in range(NPAIR):
                attention(pr, ic, pending, budget_fn)
                if ic == 0 and pr + 1 < NPAIR:
                    emit_qk(pr + 1, 0)
            pending = chunk_norm(ic)
            # drain any leftover fillers before moving to the next chunk's
            # attention (they are that chunk's dependencies)
            flush()

        for fn in pending:
            fn()
        for nt in range(NTO):
            emit_proj(nt, TCH - 1)

    if split_waits:
        _split_excess_waits(nc, 1)
    return nc


def shard_inputs(x, w_attn, b_attn, w_proj, b_proj):
    """Build the 8 per-core input dicts (core = 2*batch + head_group)."""
    x = np.asarray(x, dtype=np.float32)
    w_attn = np.asarray(w_attn, dtype=np.float32)
    b_attn = np.asarray(b_attn, dtype=np.float32)
    w_proj = np.asarray(w_proj, dtype=np.float32)
    b_proj = np.asarray(b_proj, dtype=np.float32)

    # additive causal mask for a diagonal 128x128 block of S.T ([j, i]):
    # 0 where j <= i, -1e30 above the diagonal.
    pp = np.arange(P)
    msk = np.where(pp[:, None] <= pp[None, :], 0.0, -1e30).astype(np.float32)

    def wtile(w2d, ncols):  # [C_rows, ncols] -> [P, rows//P, ncols] bf16
        r = w2d.shape[0]
        return np.ascontiguousarray(
            w2d.reshape(r // P, P, ncols).transpose(1, 0, 2)
        ).astype(NP_BF16)

    in_maps = []
    for core in range(8):
        b, hg = divmod(core, 2)
        q0 = hg * NL
        xt = np.ascontiguousarray(x[b].T)  # [C, T]
        m = {
            "xT": np.ascontiguousarray(
                xt.reshape(CT, P, TCH, 512).transpose(1, 2, 0, 3)
            ).astype(NP_BF16),
            "wq": wtile(w_attn[:, q0 : q0 + NL], NL),
            "wk": wtile(w_attn[:, C + q0 : C + q0 + NL], NL),
            "wv": wtile(w_attn[:, 2 * C + q0 : 2 * C + q0 + NL], NL),
            "wp": wtile(w_proj[q0 : q0 + NL, :], C),
            "bq": np.ascontiguousarray(
                b_attn[q0 : q0 + NL].reshape(NPAIR, P).T
            ).astype(np.float32),
            "bk": np.ascontiguousarray(
                b_attn[C + q0 : C + q0 + NL].reshape(NPAIR, P).T
            ).astype(np.float32),
            "bv": np.broadcast_to(
                b_attn[2 * C + q0 : 2 * C + q0 + NL], (P, NL)
            ).astype(np.float32),
            "bp": (
                np.ascontiguousarray(b_proj.reshape(NTO, P).T).astype(np.float32)
                if hg == 0
                else np.zeros((P, NTO), np.float32)
            ),
            "msk": msk,
        }
        in_maps.append(m)
    return in_maps


def unshard_output(results):
    """Combine 8 per-core outT [P, NTO, T] bf16 partials into [B, T, C] fp32."""
    out = np.empty((B, T, C), dtype=np.float32)
    for b in range(B):
        acc = results[2 * b]["outT"].astype(np.float32) + results[
            2 * b + 1
        ]["outT"].astype(np.float32)
        # [P, NTO, T] -> [C, T] -> [T, C]
        out[b] = acc.transpose(1, 0, 2).reshape(C, T).T
    return out


_NC_CACHE = {}


def kernel(x, w_attn, b_attn, w_proj, b_proj):
    if "nc" not in _NC_CACHE:
        _NC_CACHE["nc"] = build_nc()
    nc = _NC_CACHE["nc"]
    in_maps = shard_inputs(x, w_attn, b_attn, w_proj, b_proj)
    res = run_bass_kernel_spmd(nc, in_maps, core_ids=list(range(8)))
    return unshard_output(res.results)
